# revision 2
# baseline (speedup 1.0000x reference)
"""Trainium2 Bass kernel v2 for grouped multi-head attention.

B=16, S=7500, H=64; frames T=300, J=25 joint groups, hs=4 heads, dk=64.
Folded weights: A_h = Wq_h Wk_h^T * dk^-0.5 (64x64), G_h = Wv_h Wo_h (64x64).
Per (b,j,h):  scT (s,t) = k A_h^T q^T ; p = exp(scT); wT = [v|1]^T p;
rb = 1/rowsum (DVE recip_approx_fast on psum row 64 -> gpsimd broadcast);
wTn = wT * rb ; fin (d2,t) += G_h^T wTn  (accumulated over h in psum).

All matmul operands fp16 (1 cyc/row on PE + FWL weight loads); psum fp32;
exp on ACT (psum->sbuf, fp16 out).  q/k transposed on DEVICE via xbar
transpose-DMA -- host does only astype(fp16) + views.
Sharding: batch B over 8 cores (2 per core).
PSUM: scp pool (128,2,512) bufs=2 [z + score tiles], wtp pool (128,2,512)
bufs=2 [wT per head-pair; hp0's tile doubles as the fin accumulator].
"""

import sys

for p in ("/opt/trn_rl_repo", "/root/.axon_site/_ro/trn_rl_repo"):
    if p not in sys.path:
        sys.path.insert(0, p)

import numpy as np

import concourse.bass as bass
import concourse.bacc as bacc
import concourse.mybir as mybir
import concourse.tile as tile
from concourse.bass_utils import run_bass_kernel_spmd

B, S, H = 16, 7500, 64
T, HS, DK = 300, 4, 64
J = S // T  # 25
NCORES = 8
BPC = B // NCORES  # 2
KS = [128, 128, 44]
KOFF = [0, 128, 256]
F32 = mybir.dt.float32
F16 = mybir.dt.float16

_PROG_CACHE = {}


def build_program():
    nc = bacc.Bacc(None, target_bir_lowering=False, debug=False)

    qd = nc.dram_tensor("qd", (BPC, J, 64, T), F16, kind="ExternalInput")
    kd = nc.dram_tensor("kd", (BPC, J, 64, T), F16, kind="ExternalInput")
    vd = nc.dram_tensor("vd", (BPC, J, T, H), F16, kind="ExternalInput")
    Ad = nc.dram_tensor("Ad", (64, 2, 128), F16, kind="ExternalInput")
    Gd = nc.dram_tensor("Gd", (64, HS, 64), F16, kind="ExternalInput")
    outd = nc.dram_tensor("outd", (BPC, J, 64, T), F16, kind="ExternalOutput")

    EXP = mybir.ActivationFunctionType.Exp

    with tile.TileContext(nc) as tc:
        with (
            tc.tile_pool(name="weights", bufs=1) as wpool,
            tc.tile_pool(name="io", bufs=3) as iopool,
            tc.tile_pool(name="pt", bufs=3) as ptpool,
            tc.tile_pool(name="work", bufs=3) as workpool,
            tc.tile_pool(name="scp", bufs=2, space="PSUM") as scp,
            tc.tile_pool(name="wtp", bufs=2, space="PSUM") as wtp,
        ):
            A_sb = wpool.tile([64, 2, 128], F16, tag="A")
            nc.sync.dma_start(A_sb[:], Ad[:])
            # G lives on partitions 64-127: the fin matmul runs at PE
            # row-group 64 (its rhs wTn sits at partitions 64-127).
            G_sb = wpool.tile([128, HS, 64], F16, tag="G")
            nc.sync.dma_start(G_sb[64:128], Gd[:])

            for j in range(J):
                for b in range(BPC):
                    qT_sb = iopool.tile([64, T], F16, tag="qT")
                    nc.sync.dma_start(qT_sb[:], qd[b, j])
                    # kT duplicated on both partition halves so head-odd
                    # score matmuls (rhs z at base 64) satisfy the
                    # lhsT/rhs same-base-partition rule via row-group 64.
                    kT_sb = iopool.tile([128, T], F16, tag="kT")
                    nc.sync.dma_start(kT_sb[0:64, :], kd[b, j])
                    nc.sync.dma_start(kT_sb[64:128, :], kd[b, j])
                    # PV lhsT = [ones(64) | v(64)]: psum rows 0-63 get the
                    # rowsum replicated (base 0 -- recipfast/broadcast ucode
                    # only work from base 0), rows 64-127 get the PV result.
                    v_sb = iopool.tile([128, 3, 128], F16, tag="v")
                    nc.gpsimd.memset(v_sb[:, :, 0:64], 1.0)
                    for c in range(3):
                        nc.sync.dma_start(
                            v_sb[: KS[c], c, 64:128],
                            vd[b, j, KOFF[c] : KOFF[c] + KS[c], :],
                        )

                    # z for all 4 heads: (128, hp, 300); rows 0-63 head 2hp,
                    # rows 64-127 head 2hp+1
                    z_ps = scp.tile([128, 2, 512], F32, tag="ps", name="z")
                    for hp in range(2):
                        nc.tensor.matmul(
                            z_ps[:, hp, :T], A_sb[:, hp, :], qT_sb[:],
                            start=True, stop=True,
                        )
                    zT_sb = workpool.tile([128, 2, T], F16, tag="zT")
                    nc.vector.tensor_copy(zT_sb[:], z_ps[:, :, :T])

                    fin_ps = None
                    for hp in range(2):
                        # scores: S1=[c0 hA, c0 hB] S2=[c1 hA, c1 hB]
                        # S3 bank0 = [c2 hA @0-43, c2 hB @64-107]
                        zr = [zT_sb[64 * i : 64 * i + 64, hp, :] for i in range(2)]
                        S1 = scp.tile([128, 2, 512], F32, tag="ps", name="S1")
                        S2 = scp.tile([128, 2, 512], F32, tag="ps", name="S2")
                        S3 = scp.tile([128, 2, 512], F32, tag="ps", name="S3")
                        for i in range(2):
                            ks = kT_sb[64 * i : 64 * i + 64, :]
                            nc.tensor.matmul(
                                S1[:, i, :T], ks[:, 0:128], zr[i],
                                start=True, stop=True,
                            )
                            nc.tensor.matmul(
                                S2[:, i, :T], ks[:, 128:256], zr[i],
                                start=True, stop=True,
                            )
                            nc.tensor.matmul(
                                S3[0:44, i, :T], ks[:, 256:300], zr[i],
                                start=True, stop=True,
                            )
                        pT1 = ptpool.tile([128, 2, T], F16, tag="pT1")
                        nc.scalar.activation(pT1[:], S1[:, :, :T], EXP)
                        pT2 = ptpool.tile([128, 2, T], F16, tag="pT2")
                        nc.scalar.activation(pT2[:], S2[:, :, :T], EXP)
                        pT3 = ptpool.tile([64, 2, T], F16, tag="pT3")
                        nc.scalar.activation(pT3[0:44], S3[0:44, :, :T], EXP)

                        W = wtp.tile([128, 2, 512], F32, tag="ps", name="W")
                        for i in range(2):
                            nc.tensor.matmul(
                                W[:, i, :T], v_sb[:, 0, :], pT1[:, i, :],
                                start=True, stop=False,
                            )
                            nc.tensor.matmul(
                                W[:, i, :T], v_sb[:, 1, :], pT2[:, i, :],
                                start=False, stop=False,
                            )
                            nc.tensor.matmul(
                                W[:, i, :T],
                                v_sb[0:44, 2, :],
                                pT3[0:44, i, :],
                                start=False, stop=True,
                            )
                        # normalization: recipfast on the (replicated) rowsum
                        # rows at base 0, broadcast to all 128 partitions,
                        # multiply the wT rows (64-127).
                        rb_sb = workpool.tile(
                            [128, 2, T], F32, tag=f"rb{hp}", name=f"rb{hp}"
                        )
                        for i in range(2):
                            nc.vector.reciprocal_approx_fast(
                                rb_sb[0:1, i, :], W[0:1, i, :T]
                            )
                            nc.gpsimd.partition_broadcast(
                                rb_sb[:, i, :], rb_sb[0:1, i, :], channels=128
                            )
                        wTn_sb = workpool.tile(
                            [128, 2, T], F16, tag=f"wTn{hp}", name=f"wTn{hp}"
                        )
                        nc.vector.tensor_tensor(
                            wTn_sb[64:128, :, :], W[64:128, :, :T],
                            rb_sb[64:128, :, :],
                            mybir.AluOpType.mult,
                        )
                        if hp == 0:
                            fin_ps = W  # bank 0 rows 0-63 become fin accum
                        for i in range(2):
                            h = 2 * hp + i
                            nc.tensor.matmul(
                                fin_ps[0:64, 0, :T],
                                G_sb[64:128, h, :],
                                wTn_sb[64:128, i, :],
                                start=(h == 0), stop=(h == HS - 1),
                            )
                    out_sb = workpool.tile([64, T], F16, tag="out")
                    nc.scalar.copy(out_sb[:], fin_ps[0:64, 0, :T])
                    nc.sync.dma_start(outd[b, j], out_sb[:])

    nc.compile()
    return nc


def kernel(q, k, v, Wq, Wk, Wv, Wo, _trace=False, _tmpdir=None):
    q16 = np.asarray(q, dtype=np.float16)
    k16 = np.asarray(k, dtype=np.float16)
    v16 = np.asarray(v, dtype=np.float16)
    Wq = np.asarray(Wq, dtype=np.float32)
    Wk = np.asarray(Wk, dtype=np.float32)
    Wv = np.asarray(Wv, dtype=np.float32)
    Wo = np.asarray(Wo, dtype=np.float32)

    scale = DK ** (-0.5)
    A = np.stack(
        [
            (Wq[:, 64 * h : 64 * h + 64] @ Wk[:, 64 * h : 64 * h + 64].T) * scale
            for h in range(HS)
        ]
    )  # (4, 64a, 64b)
    G = np.stack(
        [Wv[:, 64 * h : 64 * h + 64] @ Wo[64 * h : 64 * h + 64, :] for h in range(HS)]
    )  # (4, 64d, 64d2)
    Ad = np.ascontiguousarray(
        A.reshape(2, 2, 64, 64).transpose(2, 0, 1, 3).reshape(64, 2, 128)
    ).astype(np.float16)
    Gd = np.ascontiguousarray(G.transpose(1, 0, 2)).astype(np.float16)

    if "nc" not in _PROG_CACHE:
        _PROG_CACHE["nc"] = build_program()
    nc = _PROG_CACHE["nc"]

    # (B,S,H) -> (B, J, 64, T) fp16, one global transpose; per-core = views
    qT_all = np.ascontiguousarray(q16.reshape(B, J, T, H).transpose(0, 1, 3, 2))
    kT_all = np.ascontiguousarray(k16.reshape(B, J, T, H).transpose(0, 1, 3, 2))

    in_maps = []
    for core in range(NCORES):
        b0 = BPC * core
        in_maps.append(
            {
                "qd": qT_all[b0 : b0 + BPC],
                "kd": kT_all[b0 : b0 + BPC],
                "vd": v16[b0 : b0 + BPC].reshape(BPC, J, T, H),
                "Ad": Ad,
                "Gd": Gd,
            }
        )

    res = run_bass_kernel_spmd(
        nc,
        in_maps,
        core_ids=list(range(NCORES)),
        trace=_trace,
        tmpdir=_tmpdir,
    )

    out = np.empty((B, S, H), dtype=np.float32)
    for core in range(NCORES):
        o = res.results[core]["outd"]  # (BPC, J, 64, T) f16
        for b in range(BPC):
            out[BPC * core + b] = (
                o[b].transpose(0, 2, 1).reshape(S, H).astype(np.float32)
            )
    if _trace:
        return out, res
    return out


# revision 3
# speedup vs baseline: 1.0667x; 1.0667x over previous
"""Trainium2 Bass kernel v3: software-pipelined grouped multi-head attention.

Same math as v2 (folded A/G weights, [ones|v] PV lhsT, base-0 recipfast +
partition_broadcast normalization), but emission is software-pipelined per
(b, head-pair) stage: PE order per stage is
    z(cur) | PV(prev) + fin(prev) | scores(cur)
so the PE never waits on the current stage's exp (ACT) or z-cast (DVE) --
the previous stage's PV/fin fills the gap.  PSUM: scp pool of 1-bank tiles
(bufs=4) for z + the 5 score regions per stage; wtp pool (128,2,512) bufs=2
for W (per head-pair); hp0's W doubles as the fin accumulator, rows 0-63.
"""

import sys

for p in ("/opt/trn_rl_repo", "/root/.axon_site/_ro/trn_rl_repo"):
    if p not in sys.path:
        sys.path.insert(0, p)

import numpy as np

import concourse.bass as bass
import concourse.bacc as bacc
import concourse.mybir as mybir
import concourse.tile as tile
from concourse.bass_utils import run_bass_kernel_spmd

B, S, H = 16, 7500, 64
T, HS, DK = 300, 4, 64
J = S // T  # 25
NCORES = 8
BPC = B // NCORES  # 2
KS = [128, 128, 44]
KOFF = [0, 128, 256]
F32 = mybir.dt.float32
F16 = mybir.dt.float16

_PROG_CACHE = {}


def build_program():
    nc = bacc.Bacc(None, target_bir_lowering=False, debug=False)

    qd = nc.dram_tensor("qd", (BPC, J, 64, T), F16, kind="ExternalInput")
    kd = nc.dram_tensor("kd", (BPC, J, 64, T), F16, kind="ExternalInput")
    vd = nc.dram_tensor("vd", (BPC, J, T, H), F16, kind="ExternalInput")
    Ad = nc.dram_tensor("Ad", (64, 2, 128), F16, kind="ExternalInput")
    Gd = nc.dram_tensor("Gd", (64, HS, 64), F16, kind="ExternalInput")
    outd = nc.dram_tensor("outd", (BPC, J, 64, T), F16, kind="ExternalOutput")

    EXP = mybir.ActivationFunctionType.Exp

    with tile.TileContext(nc) as tc:
        with (
            tc.tile_pool(name="weights", bufs=1) as wpool,
            tc.tile_pool(name="io", bufs=3) as iopool,
            tc.tile_pool(name="pt", bufs=3) as ptpool,
            tc.tile_pool(name="work", bufs=3) as workpool,
            tc.tile_pool(name="scp", bufs=4, space="PSUM") as scp,
            tc.tile_pool(name="wtp", bufs=2, space="PSUM") as wtp,
        ):
            A_sb = wpool.tile([64, 2, 128], F16, tag="A")
            nc.sync.dma_start(A_sb[:], Ad[:])
            G_sb = wpool.tile([128, HS, 64], F16, tag="G")
            nc.sync.dma_start(G_sb[64:128], Gd[:])

            fin_tiles = {}  # b -> W tile of (b, hp0), rows 0-63 = fin accum

            def emit_pv_chain(st):
                """PV + normalize + fin (+ out at hp1) for a finished stage."""
                b, hp, jj = st["b"], st["hp"], st["j"]
                v_sb = st["v"]
                W = wtp.tile([128, 2, 512], F32, tag="W", name=f"W{hp}")
                for i in range(2):
                    nc.tensor.matmul(
                        W[:, i, :T], v_sb[:, 0, :], st["pc0"][i][:, :],
                        start=True, stop=False,
                    )
                    nc.tensor.matmul(
                        W[:, i, :T], v_sb[:, 1, :], st["pc1"][i][:, :],
                        start=False, stop=False,
                    )
                    if i == 0:
                        nc.tensor.matmul(
                            W[:, 0, :T], v_sb[0:44, 2, :], st["pc2"][0:44, :],
                            start=False, stop=True,
                        )
                    else:
                        nc.tensor.matmul(
                            W[:, 1, :T], v_sb[64:108, 2, :], st["pc2"][64:108, :],
                            start=False, stop=True,
                        )
                rb_sb = workpool.tile([128, 2, T], F32, tag=f"rb{hp}")
                for i in range(2):
                    nc.vector.reciprocal_approx_fast(
                        rb_sb[0:1, i, :], W[0:1, i, :T]
                    )
                    nc.gpsimd.partition_broadcast(
                        rb_sb[:, i, :], rb_sb[0:1, i, :], channels=128
                    )
                wTn_sb = workpool.tile([128, 2, T], F16, tag=f"wTn{hp}")
                nc.vector.tensor_tensor(
                    wTn_sb[64:128, :, :], W[64:128, :, :T], rb_sb[64:128, :, :],
                    mybir.AluOpType.mult,
                )
                if hp == 0:
                    fin_tiles[b] = W
                fin = fin_tiles[b]
                for i in range(2):
                    h = 2 * hp + i
                    nc.tensor.matmul(
                        fin[0:64, 0, :T], G_sb[64:128, h, :], wTn_sb[64:128, i, :],
                        start=(h == 0), stop=(h == HS - 1),
                    )
                if hp == 1:
                    out_sb = workpool.tile([64, T], F16, tag="out")
                    nc.scalar.copy(out_sb[:], fin[0:64, 0, :T])
                    nc.sync.dma_start(outd[b, jj], out_sb[:])
                    del fin_tiles[b]

            prev = None
            for j in range(J):
                for b in range(BPC):
                    # per-b inputs (shared by both hp stages)
                    qT_sb = iopool.tile([64, T], F16, tag="qT")
                    nc.sync.dma_start(qT_sb[:], qd[b, j])
                    # kT padded to 320 cols (zeros) so the c2 score matmul
                    # can use M=64 and fully cover the exp'd psum rows.
                    kT_sb = iopool.tile([128, 320], F16, tag="kT")
                    nc.gpsimd.memset(kT_sb[:, 300:320], 0.0)
                    nc.sync.dma_start(kT_sb[0:64, 0:T], kd[b, j])
                    nc.sync.dma_start(kT_sb[64:128, 0:T], kd[b, j])
                    v_sb = iopool.tile([128, 3, 128], F16, tag="v")
                    nc.gpsimd.memset(v_sb[:, :, 0:64], 1.0)
                    for c in range(3):
                        nc.sync.dma_start(
                            v_sb[: KS[c], c, 64:128],
                            vd[b, j, KOFF[c] : KOFF[c] + KS[c], :],
                        )
                    # c2 chunk duplicated at base 64 (head-B PV needs
                    # lhsT/rhs both at row-group 64)
                    nc.sync.dma_start(
                        v_sb[64:108, 2, 64:128], vd[b, j, 256:300, :]
                    )
                    for hp in range(2):
                        cur = {"b": b, "hp": hp, "j": j, "v": v_sb}
                        # z for this head-pair
                        z_ps = scp.tile([128, 512], F32, tag="sc", name="z")
                        nc.tensor.matmul(
                            z_ps[:, :T], A_sb[:, hp, :], qT_sb[:],
                            start=True, stop=True,
                        )
                        zT_sb = workpool.tile([128, T], F16, tag=f"zT{hp}")
                        nc.vector.tensor_copy(zT_sb[:], z_ps[:, :T])

                        # previous stage's PV/fin fills the PE while
                        # this stage's z-cast + exps run on DVE/ACT
                        if prev is not None:
                            emit_pv_chain(prev)

                        # scores for this stage
                        zr = [zT_sb[0:64, :], zT_sb[64:128, :]]
                        sc_t = []
                        for i in range(2):
                            ks = kT_sb[64 * i : 64 * i + 64, :]
                            for c in range(2):
                                St = scp.tile(
                                    [128, 512], F32, tag="sc", name=f"S{c}{i}"
                                )
                                nc.tensor.matmul(
                                    St[:, :T],
                                    ks[:, KOFF[c] : KOFF[c] + KS[c]],
                                    zr[i],
                                    start=True, stop=True,
                                )
                                sc_t.append(St)
                        S2t = scp.tile([128, 512], F32, tag="sc", name="S2")
                        for i in range(2):
                            ks = kT_sb[64 * i : 64 * i + 64, :]
                            nc.tensor.matmul(
                                S2t[64 * i : 64 * i + 64, :T],
                                ks[:, 256:320], zr[i],
                                start=True, stop=True,
                            )
                        # exps: pc0/pc1 per head, pc2 shared (rows 0-43, 64-107)
                        pc0, pc1 = [], []
                        for i in range(2):
                            p0 = ptpool.tile([128, T], F16, tag=f"pc0{i}")
                            nc.scalar.activation(p0[:], sc_t[2 * i][:, :T], EXP)
                            pc0.append(p0)
                            p1 = ptpool.tile([128, T], F16, tag=f"pc1{i}")
                            nc.scalar.activation(p1[:], sc_t[2 * i + 1][:, :T], EXP)
                            pc1.append(p1)
                        pc2 = ptpool.tile([128, T], F16, tag="pc2")
                        nc.scalar.activation(pc2[:], S2t[:, :T], EXP)
                        cur["pc0"], cur["pc1"], cur["pc2"] = pc0, pc1, pc2
                        prev = cur
            emit_pv_chain(prev)

    nc.compile()
    return nc


def kernel(q, k, v, Wq, Wk, Wv, Wo, _trace=False, _tmpdir=None):
    q16 = np.asarray(q, dtype=np.float16)
    k16 = np.asarray(k, dtype=np.float16)
    v16 = np.asarray(v, dtype=np.float16)
    Wq = np.asarray(Wq, dtype=np.float32)
    Wk = np.asarray(Wk, dtype=np.float32)
    Wv = np.asarray(Wv, dtype=np.float32)
    Wo = np.asarray(Wo, dtype=np.float32)

    scale = DK ** (-0.5)
    A = np.stack(
        [
            (Wq[:, 64 * h : 64 * h + 64] @ Wk[:, 64 * h : 64 * h + 64].T) * scale
            for h in range(HS)
        ]
    )
    G = np.stack(
        [Wv[:, 64 * h : 64 * h + 64] @ Wo[64 * h : 64 * h + 64, :] for h in range(HS)]
    )
    Ad = np.ascontiguousarray(
        A.reshape(2, 2, 64, 64).transpose(2, 0, 1, 3).reshape(64, 2, 128)
    ).astype(np.float16)
    Gd = np.ascontiguousarray(G.transpose(1, 0, 2)).astype(np.float16)

    if "nc" not in _PROG_CACHE:
        _PROG_CACHE["nc"] = build_program()
    nc = _PROG_CACHE["nc"]

    qT_all = np.ascontiguousarray(q16.reshape(B, J, T, H).transpose(0, 1, 3, 2))
    kT_all = np.ascontiguousarray(k16.reshape(B, J, T, H).transpose(0, 1, 3, 2))

    in_maps = []
    for core in range(NCORES):
        b0 = BPC * core
        in_maps.append(
            {
                "qd": qT_all[b0 : b0 + BPC],
                "kd": kT_all[b0 : b0 + BPC],
                "vd": v16[b0 : b0 + BPC].reshape(BPC, J, T, H),
                "Ad": Ad,
                "Gd": Gd,
            }
        )

    res = run_bass_kernel_spmd(
        nc,
        in_maps,
        core_ids=list(range(NCORES)),
        trace=_trace,
        tmpdir=_tmpdir,
    )

    out = np.empty((B, S, H), dtype=np.float32)
    for core in range(NCORES):
        o = res.results[core]["outd"]  # (BPC, J, 64, T) f16
        for b in range(BPC):
            out[BPC * core + b] = (
                o[b].transpose(0, 2, 1).reshape(S, H).astype(np.float32)
            )
    if _trace:
        return out, res
    return out
